# revision 33
# baseline (speedup 1.0000x reference)
"""AxialAttention3D Trainium2 kernel.

Reference computes, for each of 3 weight branches (d/h/w), a full global
multi-head attention over the flattened 16^3 = 4096 spatial positions of
x (1, 128, 16, 16, 16), with 8 heads x dim_head 16, then
    out = gamma * (out_d + out_h + out_w) + x.

Sharding: 3 branches x 8 heads = 24 independent (branch, head) attention
units.  Core c computes head c of all 3 branches (3 units/core).  Each core
returns its partial projected output (128, 4096); the host sums the 8
partials and adds the residual x.

Per-core pipeline (all matmuls fp32 data via float32r full-rate mode):
  phase 0: load x (C=128, N=4096); project q,k per unit with the weights
    replicated at 4 partition offsets (row-tiling of the K=16 scores
    matmuls); project v transposed (v^T: m on partitions) for all 3 units
    in one matmul per m-tile, with an appended ones column for the softmax
    denominators.
  phase 1 (per 512-wide query chunk, per unit, per group of 3 key tiles):
    scoresT(m,n) = k^T q in PSUM -> ACT exp -> P^T in SBUF ->
    attn@V accumulation out[d,n] (+denom row) into a per-chunk PSUM bank
    striped by unit at partitions 32u..32u+16.
  epilogue per chunk: DMA-rearrange the 3 denom rows to (96, 16), one DVE
    reciprocal, DMA back + partition-broadcast, normalize, single stacked
    out-projection matmul over all 3 units, bias, DMA out.
"""

import numpy as np


def _bf16np():
    import ml_dtypes

    return ml_dtypes.bfloat16


HEADS = 8
DH = 16
C = 128
NCORES = 8

_FULL = dict(MT=32, CHUNK=512, NCH=8, GRP=3)
_CACHE = {}


def _patch_tile_drain():
    """walrus in this env rejects >1 sync wait on one instruction; split the
    Tile kernel-tail drain's aggregated waits into one drain per wait."""
    import concourse.mybir as mybir
    from concourse.tile import TileContext, ScopedClock

    if getattr(TileContext, "_drain_split_patched", False):
        return

    def _drain_and_barrier_split(self, tick_clock, wait_clock):
        probe = self.nc.sync.drain()
        wait_clock.add_sem_waits(
            probe.ins, ScopedClock({None: tick_clock.global_clock})
        )
        si = probe.ins.sync_info
        waits = list(si.on_wait) if si is not None else []
        if len(waits) > 1:
            si.on_wait = [waits[0]]
            for w in waits[1:]:
                d = self.nc.sync.drain()
                d.ins.sync_info = mybir.SyncInfo(on_wait=[w], on_update=[])
        self.nc.all_engine_barrier()
        assert self.sems is not None
        popped = self.nc._tile_sem_poison_stack.pop()
        assert popped is self._sem_poison
        self.nc.clear_and_free_semaphores(list(self.sems.allocated().values()))
        self.nc.all_engine_barrier()

    TileContext._drain_and_barrier = _drain_and_barrier_split
    TileContext._drain_split_patched = True


def _split_multi_waits(nc):
    """walrus in this env allows at most ONE sync wait per instruction.
    Hoist extra waits onto same-engine NoOps inserted just before."""
    import concourse.mybir as mybir

    for f in nc.m.functions:
        for bb in f.blocks:
            new = []
            changed = False
            for inst in bb.instructions:
                si = inst.sync_info
                if si is not None and si.on_wait and len(si.on_wait) > 1:
                    waits = list(si.on_wait)
                    for j, w in enumerate(waits[:-1]):
                        nop = mybir.InstNoOp(
                            name=f"{inst.name}-w{j}",
                            engine=inst.engine,
                            sync_info=mybir.SyncInfo(on_wait=[w], on_update=[]),
                            bass_nofuse=True,
                        )
                        new.append(nop)
                    si.on_wait = [waits[-1]]
                    changed = True
                new.append(inst)
            if changed:
                bb.instructions = new


def build_nc(cfg=_FULL, split_waits=True):
    import concourse.bass as bass
    import concourse.mybir as mybir
    from concourse import tile, library_config

    _patch_tile_drain()

    f32 = mybir.dt.float32
    f32r = mybir.dt.float32r
    bf16 = mybir.dt.bfloat16
    Exp = mybir.ActivationFunctionType.Exp

    MT, CHUNK, NCH, GRP = cfg["MT"], cfg["CHUNK"], cfg["NCH"], cfg["GRP"]
    N = MT * 128
    assert N == CHUNK * NCH
    SUB = CHUNK // 32  # free width of the rearranged denominator block

    nc = bass.Bass("TRN2", target_bir_lowering=False, debug=False)

    x_d = nc.declare_dram_parameter("x", [C, N], bf16, isOutput=False)
    lq_d = [
        nc.declare_dram_parameter(f"lq{u}", [C, 128], bf16, isOutput=False)
        for u in range(3)
    ]
    lk_d = [
        nc.declare_dram_parameter(f"lk{u}", [C, 128], bf16, isOutput=False)
        for u in range(3)
    ]
    bq_d = [
        nc.declare_dram_parameter(f"bq{u}", [C, 1], f32, isOutput=False)
        for u in range(3)
    ]
    bk_d = [
        nc.declare_dram_parameter(f"bk{u}", [C, 1], f32, isOutput=False)
        for u in range(3)
    ]
    wv_d = nc.declare_dram_parameter("wv3", [C, 52], bf16, isOutput=False)
    wo_d = nc.declare_dram_parameter("wo", [C, 128], f32r, isOutput=False)
    be_d = nc.declare_dram_parameter("beff", [C, 1], f32, isOutput=False)
    onesv_d = nc.declare_dram_parameter("onesv", [C, 96], bf16, isOutput=False)
    zeros_d = nc.declare_dram_parameter("zerosc", [C, CHUNK], f32r, isOutput=False)
    y_d = nc.declare_dram_parameter("y", [C, N], f32, isOutput=True)

    with tile.TileContext(nc) as tc:
        with (
            tc.tile_pool(name="persist", bufs=1) as pp,
            tc.tile_pool(name="pt", bufs=4) as ptp,
            tc.tile_pool(name="osb", bufs=2) as osbp,
            tc.tile_pool(name="big", bufs=2, space="PSUM") as bigp,
            tc.tile_pool(name="accp", bufs=1, space="PSUM") as accp,
            tc.tile_pool(name="projp", bufs=1, space="PSUM") as projp,
        ):
            # ---- persistent SBUF tensors ----
            x_sb = pp.tile([C, N], bf16, name="x_sb", tag="x")
            nc.sync.dma_start(x_sb[:], x_d[:])
            lq = [pp.tile([C, 128], bf16, name=f"lq{u}_sb", tag=f"lq{u}") for u in range(3)]
            lk = [pp.tile([C, 128], bf16, name=f"lk{u}_sb", tag=f"lk{u}") for u in range(3)]
            bq = [pp.tile([C, 1], f32, name=f"bq{u}_sb", tag=f"bq{u}") for u in range(3)]
            bk = [pp.tile([C, 1], f32, name=f"bk{u}_sb", tag=f"bk{u}") for u in range(3)]
            for u in range(3):
                nc.sync.dma_start(lq[u][:], lq_d[u][:])
                nc.sync.dma_start(lk[u][:], lk_d[u][:])
                nc.sync.dma_start(bq[u][:], bq_d[u][:])
                nc.sync.dma_start(bk[u][:], bk_d[u][:])
            wv = pp.tile([C, 52], bf16, name="wv_sb", tag="wv")
            wo = pp.tile([C, 128], f32r, name="wo_sb", tag="wo")
            be = pp.tile([C, 1], f32, name="be_sb", tag="be")
            nc.sync.dma_start(wv[:], wv_d[:])
            nc.sync.dma_start(wo[:], wo_d[:])
            nc.sync.dma_start(be[:], be_d[:])

            qrep = [pp.tile([C, N], bf16, name=f"q{u}_sb", tag=f"q{u}") for u in range(3)]
            krep = [pp.tile([C, N], bf16, name=f"k{u}_sb", tag=f"k{u}") for u in range(3)]
            vT = pp.tile([C, MT * 51], bf16, name="vT_sb", tag="vT")
            denb = pp.tile([C, SUB], f32, name="denb_sb", tag="denb")
            recb = pp.tile([C, SUB], f32, name="recb_sb", tag="recb")
            dstage = pp.tile([C, CHUNK], f32, name="dstage_sb", tag="dstage")
            normsb = pp.tile([C, CHUNK], f32, name="normsb_sb", tag="normsb")
            recrow = pp.tile([C, CHUNK], f32, name="recrow_sb", tag="recrow")
            scaled = pp.tile([C, CHUNK], f32r, name="scaled_sb", tag="scaled")

            nc.sync.dma_start(scaled[:], zeros_d[:])

            # ---- phase 0 (emitted partly up-front, partly dripped into
            # the phase-1 pipeline so the ACT exp stream starts early) ----
            def emit_vt(t):
                ps = bigp.tile([C, 52], f32, name="vps", tag="scores")
                nc.tensor.matmul(
                    ps[:],
                    lhsT=x_sb[:, t * 128 : (t + 1) * 128],
                    rhs=wv[:],
                    start=True,
                    stop=True,
                )
                nc.vector.tensor_copy(vT[:, t * 51 : t * 51 + 51], ps[:, 0:51])
                ones_ap = vT[:, t * 51 : t * 51 + 51].rearrange(
                    "p (u d) -> p u d", d=17
                )[:, :, 16]
                nc.vector.memset(ones_ap, 1.0)

            def emit_qk(u, cidx):
                cs, ce = cidx * CHUNK, (cidx + 1) * CHUNK
                psq = bigp.tile([C, CHUNK], f32, name="qkps", tag="scores")
                nc.tensor.matmul(
                    psq[:], lhsT=lq[u][:], rhs=x_sb[:, cs:ce], start=True, stop=True
                )
                nc.vector.tensor_scalar_add(qrep[u][:, cs:ce], psq[:], bq[u][:])
                psk = bigp.tile([C, CHUNK], f32, name="qkps", tag="scores")
                nc.tensor.matmul(
                    psk[:], lhsT=lk[u][:], rhs=x_sb[:, cs:ce], start=True, stop=True
                )
                nc.vector.tensor_scalar_add(krep[u][:, cs:ce], psk[:], bk[u][:])

            # Pre-block: everything unit 0's chunk-0 score sweep touches
            # (its 11 groups read ALL of krep[u0] within the first 11 slots)
            # plus the first vT tiles.  The rest is dripped into the
            # pipeline at 3 ops/slot, vT first (attn@V needs tile t by slot
            # ~t/3), then u1/u2 projections (needed from slots 12/23 on).
            NPRE = min(8, MT)
            for t in range(NPRE):
                emit_vt(t)
            for cidx in range(NCH):
                emit_qk(0, cidx)
            emit_qk(1, 0)
            emit_qk(2, 0)
            drip = (
                [("vt", t) for t in range(NPRE, MT)]
                + [("qk", (1, cidx)) for cidx in range(1, NCH)]
                + [("qk", (2, cidx)) for cidx in range(1, NCH)]
            )

            # ---- phase 1: attention (software-pipelined PE stream) ----
            # The PE queue is strictly in-order: any instruction whose input
            # is not ready stalls it, and >1us stalls re-throttle the PE
            # clock to 1.2 GHz (HAM).  So: emit attn@V with a LAG-group
            # delay (its exp() is then long finished) and defer each
            # chunk's projection matmul into the middle of the next chunk.
            groups = []
            t0 = 0
            while t0 < MT:
                groups.append(list(range(t0, min(t0 + GRP, MT))))
                t0 += GRP
            LAG = 2
            EPI_DELAY = 3  # group-slots after a chunk's last attn@V

            items = []
            for cidx in range(NCH):
                for u in range(3):
                    for tlist in groups:
                        items.append((cidx, u, tlist))
            n_items = len(items)
            per_chunk = 3 * len(groups)

            acc_of_chunk = {}
            pt_of_item = {}

            def emit_scores(idx):
                cidx, u, tlist = items[idx]
                cs, ce = cidx * CHUNK, (cidx + 1) * CHUNK
                sc = bigp.tile(
                    [C, CHUNK * len(tlist)], f32, name="sc_ps", tag="scores"
                )
                for i, t in enumerate(tlist):
                    r = t % 4
                    nc.tensor.matmul(
                        sc[:, i * CHUNK : (i + 1) * CHUNK],
                        lhsT=krep[u][32 * r : 32 * r + 16, t * 128 : (t + 1) * 128],
                        rhs=qrep[u][32 * r : 32 * r + 16, cs:ce],
                        start=True,
                        stop=True,
                        tile_position=(32 * r, 0),
                    )
                pt = ptp.tile([C, CHUNK * len(tlist)], bf16, name="pt_sb", tag="pt")
                nc.scalar.activation(pt[:], sc[:], Exp)
                pt_of_item[idx] = pt

            def emit_attnv(idx):
                cidx, u, tlist = items[idx]
                if cidx not in acc_of_chunk:
                    acc_of_chunk[cidx] = accp.tile(
                        [C, CHUNK], f32, name="acc_ps", tag="acc"
                    )
                acc = acc_of_chunk[cidx]
                pt = pt_of_item.pop(idx)
                for i, t in enumerate(tlist):
                    nc.tensor.matmul(
                        acc[32 * u : 32 * u + 17, :],
                        lhsT=vT[:, 51 * t + 17 * u : 51 * t + 17 * u + 17],
                        rhs=pt[:, i * CHUNK : (i + 1) * CHUNK],
                        start=(t == 0),
                        stop=(t == MT - 1),
                    )

            def emit_epilogue_a(cidx):
                # denominators -> reciprocal -> broadcast -> normalize
                # (no PE instructions; runs on DVE/DMA alongside next chunk)
                acc = acc_of_chunk.pop(cidx)
                for u in range(3):
                    nc.vector.tensor_copy(
                        dstage[32 * u : 32 * u + 17, :],
                        acc[32 * u : 32 * u + 17, :],
                    )
                for u in range(3):
                    nc.sync.dma_start(
                        denb[32 * u : 32 * u + 32, :],
                        dstage[32 * u + 16 : 32 * u + 17, :],
                    )
                nc.vector.reciprocal(recb[0:96, :], denb[0:96, :])
                for u in range(3):
                    base = 32 * u
                    nc.sync.dma_start(
                        normsb[base : base + 1, :], recb[base : base + 32, :]
                    )
                    # log-doubling broadcast to rows base..base+16
                    for w in (1, 2, 4, 8):
                        nc.sync.dma_start(
                            normsb[base + w : base + 2 * w, :],
                            normsb[base : base + w, :],
                        )
                    nc.sync.dma_start(
                        normsb[base + 16 : base + 17, :], normsb[base : base + 1, :]
                    )
                for u in range(3):
                    nc.vector.tensor_mul(
                        scaled[32 * u : 32 * u + 17, :],
                        dstage[32 * u : 32 * u + 17, :],
                        normsb[32 * u : 32 * u + 17, :],
                    )

            def emit_epilogue_b(cidx):
                cs, ce = cidx * CHUNK, (cidx + 1) * CHUNK
                pj = projp.tile([C, CHUNK], f32, name="pj_ps", tag="proj")
                nc.tensor.matmul(
                    pj[:], lhsT=wo[:], rhs=scaled[:], start=True, stop=True
                )
                osb = osbp.tile([C, CHUNK], f32, name="osb_sb", tag="osb")
                nc.vector.tensor_scalar_add(osb[:], pj[:], be[:])
                nc.sync.dma_start(y_d[:, cs:ce], osb[:])

            pending_b = []
            for idx in range(n_items + LAG + EPI_DELAY + 1):
                while pending_b and pending_b[0][0] <= idx:
                    emit_epilogue_b(pending_b.pop(0)[1])
                for _ in range(3):
                    if drip:
                        kind, arg = drip.pop(0)
                        if kind == "vt":
                            emit_vt(arg)
                        else:
                            emit_qk(*arg)
                if idx < n_items:
                    emit_scores(idx)
                av = idx - LAG
                if 0 <= av < n_items:
                    emit_attnv(av)
                    if (av + 1) % per_chunk == 0:
                        c = av // per_chunk
                        # epilogue_b(c-1) must precede epilogue_a(c): both
                        # use the shared `scaled` buffer
                        while pending_b:
                            emit_epilogue_b(pending_b.pop(0)[1])
                        emit_epilogue_a(c)
                        pending_b.append((idx + EPI_DELAY, c))
            while pending_b:
                emit_epilogue_b(pending_b.pop(0)[1])

    if split_waits:
        _split_multi_waits(nc)
    return nc


def host_prep(inputs, cfg=_FULL):
    """Slice/pack the full problem inputs into per-core input maps."""
    MT, CHUNK, NCH = cfg["MT"], cfg["CHUNK"], cfg["NCH"]
    N = MT * 128

    x = np.asarray(inputs["x"], dtype=np.float32)
    B = x.shape[0]
    assert B == 1
    xf = np.ascontiguousarray(x.reshape(C, -1))[:, :N]

    gamma0 = float(np.asarray(inputs["gamma"]).reshape(-1)[0])
    branches = [
        (
            np.asarray(inputs[f"w_qkv_{nm}"], dtype=np.float32),
            np.asarray(inputs[f"b_qkv_{nm}"], dtype=np.float32),
            np.asarray(inputs[f"w_out_{nm}"], dtype=np.float32),
            np.asarray(inputs[f"b_out_{nm}"], dtype=np.float32),
        )
        for nm in ("d", "h", "w")
    ]

    beff_total = np.zeros(C, dtype=np.float64)
    for wqkv, bqkv, wout, bout in branches:
        bv = bqkv[2 * C : 3 * C]
        beff_total += gamma0 * (wout.astype(np.float64) @ bv + bout)
    beff_core = (beff_total / NCORES).astype(np.float32).reshape(C, 1)

    in_maps = []
    for h in range(NCORES):
        m = {
            "x": xf.astype(_bf16np()),
            "wv3": None,
            "wo": None,
            "beff": beff_core,
            "onesv": np.ones((C, 96), dtype=_bf16np()),
            "zerosc": np.zeros((C, CHUNK), dtype=np.float32),
        }
        wv3 = np.zeros((C, 52), dtype=np.float32)
        wo_stacked = np.zeros((C, 128), dtype=np.float32)
        for u, (wqkv, bqkv, wout, bout) in enumerate(branches):
            wq = wqkv[h * DH : (h + 1) * DH, :]  # (16, 128)
            wk = wqkv[C + h * DH : C + (h + 1) * DH, :]
            wvu = wqkv[2 * C + h * DH : 2 * C + (h + 1) * DH, :]
            bqu = bqkv[h * DH : (h + 1) * DH]
            bku = bqkv[C + h * DH : C + (h + 1) * DH]

            lqm = np.zeros((C, 128), dtype=np.float32)
            lkm = np.zeros((C, 128), dtype=np.float32)
            bqm = np.zeros((C, 1), dtype=np.float32)
            bkm = np.zeros((C, 1), dtype=np.float32)
            for r in range(4):
                lqm[:, 32 * r : 32 * r + 16] = 0.5 * wq.T
                lkm[:, 32 * r : 32 * r + 16] = 0.5 * wk.T
                bqm[32 * r : 32 * r + 16, 0] = 0.5 * bqu
                bkm[32 * r : 32 * r + 16, 0] = 0.5 * bku
            m[f"lq{u}"] = lqm.astype(_bf16np())
            m[f"lk{u}"] = lkm.astype(_bf16np())
            m[f"bq{u}"] = bqm
            m[f"bk{u}"] = bkm

            wv3[:, u * 17 : u * 17 + 16] = wvu.T  # col 16 stays 0
            wo_stacked[32 * u : 32 * u + 16, :] = gamma0 * wout[:, h * DH : (h + 1) * DH].T
        m["wv3"] = wv3.astype(_bf16np())
        m["wo"] = wo_stacked
        in_maps.append(m)
    return in_maps


def gather(results, inputs, cfg=_FULL):
    x = np.asarray(inputs["x"], dtype=np.float32)
    N = cfg["MT"] * 128
    acc = np.zeros((C, N), dtype=np.float32)
    for r in results:
        acc += r["y"]
    out = acc + x.reshape(C, -1)[:, :N]
    return out.reshape(x.shape).astype(np.float32)


def kernel(**inputs) -> np.ndarray:
    from concourse.bass_utils import run_bass_kernel_spmd

    if "nc" not in _CACHE:
        _CACHE["nc"] = build_nc(_FULL)
    nc = _CACHE["nc"]
    in_maps = host_prep(inputs, _FULL)
    res = run_bass_kernel_spmd(nc, in_maps, list(range(NCORES)))
    return gather(res.results, inputs, _FULL)
